# revision 4
# baseline (speedup 1.0000x reference)
# Bicycle-model trajectory rollout on 8 Trainium2 NeuronCores (Bass/Tile).
#
# Math (per trajectory, 255 steps):
#   sp'  = relu(sp + DT*(a - (sp*0.1 + (0.01*sp)*sp)))   # upper clip at 100 never binds
#   yaw' = yaw + sp*tan(clip(st))*(DT/W)
#   x'   = x + (sp*DT)*cos(yaw) ;  y' similarly with sin
#
# Structure (per core: 8192 trajectories = 128 partitions x 64, T=256):
#  - phase B: speed recurrence; two half-batch custom-DVE ops per step
#    (FD=32, alternating halves hide the read-after-write ack bubble).
#  - tan(clip(st))*(DT/W) in ONE 6-stage custom DVE op (cubic minimax fit,
#    max err 8e-4) written into a 257-slot frame whose slot 0 carries the
#    yaw seed.
#  - yaw/x/y cumsums via PROD_CUMSUM: a custom DVE op fusing the product
#    and the inclusive scan (scan() spec node, ~1.1 cyc/elem), one op per
#    trajectory page.  Seeds ride in slot 0 (sp frame slot 0 = 1.0, partner
#    frame slot 0 = seed/scale).  This keeps GPSIMD completely idle --
#    important because 2-stream DVE ops lock GPSIMD out of their shared
#    SBUF port (measured: gp products + DVE scans fully serialize).
#  - range reduction (magic-round frac) on DVE writes straight into the
#    sin/cos frame; ScalarE runs Sin in place (out == in works).
#  - group pipeline is software-pipelined with per-stage iteration lags so
#    NO cross-engine dependency is ever same-iteration.
#
# Measured (For_i loop, per-iteration): ~218 us vs 527 us baseline.
import sys
import os

sys.path.insert(0, "/opt/trn_rl_repo")

import numpy as np

os.environ.setdefault("JAX_COMPILATION_CACHE_DIR", "/tmp/jax_cache")
try:
    import jax
    jax.config.update("jax_compilation_cache_dir", "/tmp/jax_cache")
    jax.config.update("jax_persistent_cache_min_entry_size_bytes", -1)
    jax.config.update("jax_persistent_cache_min_compile_time_secs", 0.0)
except Exception:
    pass

import concourse.bass as bass  # noqa: F401
import concourse.tile as tile
from concourse import bacc, mybir
from concourse import dve_ops
from concourse.dve_spec import Spec, Src0, Src1, C0, C1, C2, Zero, relu, sq, maxx, minn
from concourse.bass_utils import run_bass_kernel_spmd

F32 = mybir.dt.float32
Alu = mybir.AluOpType
Act = mybir.ActivationFunctionType

N_CORES = 8
B = 65536
T = 256
BC = B // N_CORES          # 8192 trajectories per core
P = 128                    # partitions
J = BC // P                # 64 trajectories per partition
FRAME = T + 1              # 257 slots per trajectory frame
JG = 4                     # trajectories per processing group
NGROUPS = J // JG          # 16
GL = JG * FRAME            # 1028 flat slots per group frame
TC = 32                    # accel time-chunk width for phase B
DT = 0.05
WHEEL = 2.7
KY = float(np.float32(DT / WHEEL))
MAX_STEER = float(np.deg2rad(30.0))
TAN_C3 = 0.36874           # minimax fit: tan(y) ~ y + c3*y^3 on [0, MAX_STEER]
MAGIC = 1.5 * 2.0 ** 23
INV_2PI = float(np.float32(1.0 / (2 * np.pi)))
# 2*pi rounded one ulp toward zero so scale*q never exceeds the Sin domain.
SCALE_2PI = float(np.nextafter(np.float32(2 * np.pi), np.float32(0.0)))


def _register_dve_op(name, spec):
    if name in dve_ops.CUSTOM_DVE_SPECS:
        return next(op for op in dve_ops.OPS if op.name == name)
    op = dve_ops.DveOp(name, spec, False, {})
    dve_ops.OPS.append(op)
    dve_ops.CUSTOM_DVE_SPECS[name] = spec
    dve_ops._SUB_OPCODE_FOR_NAME[name] = (
        dve_ops._CUSTOM_DVE_ROW_BASE + len(dve_ops.OPS) - 1
    )
    import re

    for ver in ("v3", "v4"):
        try:
            op.compile(ver)
        except ValueError as e:
            op.uops_sha[ver] = re.search(r"([0-9a-f]{16})", str(e)).group(1)
            op.compile(ver)
    return op


# Speed step in the reference's exact fp32 op order:
#   relu(sp + (a - (sp*C0 + (C1*sp)*sp)) * C2),  C0=0.1 C1=0.01 C2=0.05
def _ref_bstep(in0, in1, c0, c1, c2):
    f = np.float32
    fr = (in0 * f(c0) + (f(c1) * in0) * in0).astype(np.float32)
    u = (in0 + (in1 - fr) * f(c2)).astype(np.float32)
    return np.maximum(np.nan_to_num(u, nan=0.0), 0)


BSTEP = _register_dve_op(
    "BICY_STEP_X",
    Spec(
        body=relu(Src0 + (Src1 - (Src0 * C0 + (C1 * Src0) * Src0)) * C2),
        reference=_ref_bstep,
    ),
)
BSTEP_CONSTS = (0.1, 0.01, 0.05)

# tan(clip(x, +-C0)) * K as a cubic:  out = (y*y*C1 + C2) * y,
# y = clip(x), C1 = K*TAN_C3, C2 = K.
_y = maxx(minn(Src0, C0), Zero - C0)


def _ref_tankc(in0, in1, c0, c1, c2):
    y = np.clip(in0, -np.float32(c0), np.float32(c0)).astype(np.float32)
    return ((y * y * np.float32(c1) + np.float32(c2)) * y).astype(np.float32)


TANKC = _register_dve_op(
    "TAN_CLIP_SC",
    Spec(body=(sq(_y) * C1 + C2) * _y, reference=_ref_tankc),
)

# q = t2 - round(t2), t2 = x*C0 + C1  (C2 = magic rounding constant).
# Sin(SCALE_2PI * q) then gives sin (C1=0) / cos (C1=0.25) of x*2pi*C0.
_t2 = Src0 * C0 + C1
REDFRAC = _register_dve_op(
    "REDUCE_FRAC_X",
    Spec(
        body=_t2 - ((_t2 + C2) - C2),
        reference=lambda in0, in1, c0, c1, c2: (
            lambda t2: (t2 - ((t2 + np.float32(c2)) - np.float32(c2)).astype(np.float32)).astype(np.float32)
        )((in0 * np.float32(c0) + np.float32(c1)).astype(np.float32)),
    ),
)

# Fused per-trajectory product + inclusive cumsum:
#   out[k] = sum_{i<=k} in0[i]*in1[i]*C2   (one 257-slot page per op)
from concourse.dve_spec import scan as _dscan, AluOp as _DAlu


def _ref_pscan(in0, in1, c0, c1, c2):
    return np.cumsum((in0 * in1).astype(np.float32) * np.float32(c2),
                     axis=-1).astype(np.float32)


PSCAN = _register_dve_op(
    "PROD_CUMSUM",
    Spec(body=_dscan(_DAlu.ADD, (Src0 * Src1) * C2), reference=_ref_pscan),
)

_BUILD_CACHE = {}


def build_kernel(reps=1, loop=False, io_internal=False, phb_halves=True,
                 do_phaseb=True, do_groups=True, do_outdma=True):
    """Build + compile the per-core program. With loop=True the body sits in a
    hardware For_i executed `reps` times. io_internal=True makes the big
    tensors Internal DRAM (zero-filled once before the loop) with a tiny
    external output, so timing runs move almost no data over the host link."""
    key = (reps, loop, io_internal, phb_halves, do_phaseb, do_groups, do_outdma)
    if key in _BUILD_CACHE:
        return _BUILD_CACHE[key]

    nc = bacc.Bacc(None, target_bir_lowering=False, debug=False)

    kin = "Internal" if io_internal else "ExternalInput"
    kout = "Internal" if io_internal else "ExternalOutput"
    d_sx = nc.dram_tensor("start_x", [BC], F32, kind="ExternalInput").ap()
    d_sy = nc.dram_tensor("start_y", [BC], F32, kind="ExternalInput").ap()
    d_syaw = nc.dram_tensor("start_yaw", [BC], F32, kind="ExternalInput").ap()
    d_ssp = nc.dram_tensor("start_speed", [BC], F32, kind="ExternalInput").ap()
    d_acc = nc.dram_tensor("accel", [BC, T], F32, kind=kin).ap()
    d_st = nc.dram_tensor("steering", [BC, T], F32, kind=kin).ap()
    d_ox = nc.dram_tensor("out_x", [BC, T], F32, kind=kout).ap()
    d_oy = nc.dram_tensor("out_y", [BC, T], F32, kind=kout).ap()
    d_oyaw = nc.dram_tensor("out_yaw", [BC, T], F32, kind=kout).ap()
    d_osp = nc.dram_tensor("out_speed", [BC, T], F32, kind=kout).ap()
    d_done = (
        nc.dram_tensor("done", [P, 4], F32, kind="ExternalOutput").ap()
        if io_internal
        else None
    )

    acc3 = d_acc.rearrange("(p j) t -> p j t", p=P)
    st3 = d_st.rearrange("(p j) t -> p j t", p=P)
    ox3 = d_ox.rearrange("(p j) t -> p j t", p=P)
    oy3 = d_oy.rearrange("(p j) t -> p j t", p=P)
    oyaw3 = d_oyaw.rearrange("(p j) t -> p j t", p=P)
    osp3 = d_osp.rearrange("(p j) t -> p j t", p=P)
    sx2 = d_sx.rearrange("(p j) -> p j", p=P)
    sy2 = d_sy.rearrange("(p j) -> p j", p=P)
    syaw2 = d_syaw.rearrange("(p j) -> p j", p=P)
    ssp2 = d_ssp.rearrange("(p j) -> p j", p=P)

    c0, c1, c2 = BSTEP_CONSTS

    with tile.TileContext(nc) as tc:
        import contextlib

        with contextlib.ExitStack() as ctx:
            p_sp = ctx.enter_context(tc.tile_pool(name="p_sp", bufs=1))
            p_const = ctx.enter_context(tc.tile_pool(name="p_const", bufs=1))
            p_acc = ctx.enter_context(tc.tile_pool(name="p_acc", bufs=2))
            p_stg = ctx.enter_context(tc.tile_pool(name="p_stg", bufs=3))
            p_tan = ctx.enter_context(tc.tile_pool(name="p_tan", bufs=3))
            p_yfr = ctx.enter_context(tc.tile_pool(name="p_yfr", bufs=3))
            p_xfr = ctx.enter_context(tc.tile_pool(name="p_xfr", bufs=3))
            p_yfr2 = ctx.enter_context(tc.tile_pool(name="p_yfr2", bufs=3))
            p_sc = ctx.enter_context(tc.tile_pool(name="p_sc", bufs=4))
            

            # ---- one-time tiles ----
            sp_st = p_sp.tile([P, J, FRAME], F32, name="sp_st")
            t_sx = p_const.tile([P, J], F32, name="t_sx")
            nc.sync.dma_start(t_sx[:], sx2[:])
            t_sy = p_const.tile([P, J], F32, name="t_sy")
            nc.sync.dma_start(t_sy[:], sy2[:])
            t_syaw = p_const.tile([P, J], F32, name="t_syaw")
            nc.sync.dma_start(t_syaw[:], syaw2[:])
            t_ssp = p_const.tile([P, J], F32, name="t_ssp")
            nc.sync.dma_start(t_ssp[:], ssp2[:])
            # x/y seeds stacked for the one-copy xy-frame seed fill
            # [yseed, xseed]/DT stacked: slot-0 values for the sin/cos
            # frames (x uses cos page 1, y uses sin page 0)
            t_sxyd = p_const.tile([P, 2, J], F32, name="t_sxyd")
            nc.vector.tensor_scalar(t_sxyd[:, 0, :], t_sy[:], 1.0 / DT, None, Alu.mult)
            nc.vector.tensor_scalar(t_sxyd[:, 1, :], t_sx[:], 1.0 / DT, None, Alu.mult)
            nc.vector.memset(sp_st[:, :, 0], 1.0)
            if not do_phaseb:
                nc.vector.memset(sp_st[:], 1.0)

            if io_internal:
                # Zero-fill the Internal accel/steering once so the timed loop
                # computes on sane values (NaN/denormal-free).
                zt = p_const.tile([P, 2048], F32, name="zt")
                nc.vector.memset(zt[:], 0.0)
                zt3 = zt.rearrange("p (j t) -> p j t", t=T)
                for k in range(J // 8):
                    nc.sync.dma_start(acc3[:, 8 * k : 8 * k + 8, :], zt3[:])
                    nc.sync.dma_start(st3[:, 8 * k : 8 * k + 8, :], zt3[:])

            import contextlib as _ctxlib

            def _loop_cm():
                if loop:
                    return tc.For_i(0, reps, 1, hint_engines=(mybir.EngineType.DVE,))
                return _ctxlib.nullcontext(iter(range(reps)))

            with _loop_cm() as _it:
                _unused = _it
                # steering prefetch for the first groups (independent of
                # phase B; keeps the group ramp off the sp-out DMA's tail)
                stg_t = {}
                if do_groups:
                    for g in range(3):
                        stg = p_stg.tile([P, JG, T], F32, name="stg")
                        nc.sync.dma_start(stg[:], st3[:, g * JG : (g + 1) * JG, :])
                        stg_t[g] = stg
                # ---- phase B: speed recurrence over all trajectories ----
                if do_phaseb:
                    nc.vector.tensor_copy(sp_st[:, :, 1], t_ssp[:])
                    acc_tiles = []
                    for c in range(T // TC):
                        at = p_acc.tile([P, J, TC], F32, name="acc")
                        nc.sync.dma_start(at[:], acc3[:, :, c * TC : (c + 1) * TC])
                        acc_tiles.append(at)
                    H = J // 2
                    for t in range(1, T):
                        ch, col = (t - 1) // TC, (t - 1) % TC
                        if phb_halves:
                            for h in (0, 1):
                                js = slice(h * H, (h + 1) * H)
                                nc.vector._custom_dve(
                                    BSTEP,
                                    out=sp_st[:, js, t + 1],
                                    in0=sp_st[:, js, t],
                                    in1=acc_tiles[ch][:, js, col],
                                    s0=c0, s1=c1, imm2=c2,
                                )
                        else:
                            nc.vector._custom_dve(
                                BSTEP,
                                out=sp_st[:, :, t + 1],
                                in0=sp_st[:, :, t],
                                in1=acc_tiles[ch][:, :, col],
                                s0=c0, s1=c1, imm2=c2,
                            )

                    # speed output (single full-row DMA; overlaps group ramp)
                    if do_outdma:
                        nc.sync.dma_start(osp3[:, :, :], sp_st[:, :, 1:FRAME])

                # ---- group pipeline (V3): fused product+cumsum custom DVE
                # ops per trajectory page; no GPSIMD, no masks.  Frames are
                # col-aligned: slot s = output column s; slot 0 = seed.
                # stage -> iteration for group g (all cross-engine edges lag>=1):
                #   stg dma: g | tan-frame: g+1 | yaw pscans: g+2
                #   qs/qc -> sincos slots 1..256 + slot0 seeds + yaw-out: g+3
                #   ACT sin in-place: g+4 | x/y pscans: g+5 | x/y out: g+6
                tan_t, yfr_t, sc_t, xfr_t, yfr2_t = {}, {}, {}, {}, {}

                def sl(g):
                    return slice(g * JG, (g + 1) * JG)

                for i in range(NGROUPS + 7 if do_groups else 0):
                    # S1: tan-frame of group i-1 (slots 1..256 + yaw seed at 0)
                    if 0 <= i - 1 < NGROUPS:
                        g = i - 1
                        tanc = p_tan.tile([P, JG, FRAME], F32, name="tanc")
                        nc.vector._custom_dve(
                            TANKC, out=tanc[:, :, 1:FRAME], in0=stg_t[g][:],
                            s0=MAX_STEER, s1=KY * TAN_C3, imm2=KY,
                        )
                        nc.vector.tensor_copy(tanc[:, :, 0], t_syaw[:, sl(g)])
                        tan_t[g] = tanc
                    # S2: yaw pscans of group i-2
                    if 0 <= i - 2 < NGROUPS:
                        g = i - 2
                        yfr = p_yfr.tile([P, JG, FRAME], F32, name="yfr")
                        for j in range(JG):
                            nc.vector._custom_dve(
                                PSCAN, out=yfr[:, j, :],
                                in0=sp_st[:, g * JG + j, :],
                                in1=tan_t[g][:, j, :],
                                s0=0.0, s1=0.0, imm2=1.0,
                            )
                        yfr_t[g] = yfr
                    # S3: range reduction into sincos frame + seeds + yaw out
                    if 0 <= i - 3 < NGROUPS:
                        g = i - 3
                        sincos = p_sc.tile([P, 2, JG, FRAME], F32, name="sincos")
                        nc.vector._custom_dve(
                            REDFRAC, out=sincos[:, 0, :, 1:FRAME],
                            in0=yfr_t[g][:, :, 0:T],
                            s0=INV_2PI, s1=0.0, imm2=MAGIC,
                        )
                        nc.vector._custom_dve(
                            REDFRAC, out=sincos[:, 1, :, 1:FRAME],
                            in0=yfr_t[g][:, :, 0:T],
                            s0=INV_2PI, s1=0.25, imm2=MAGIC,
                        )
                        nc.vector.tensor_copy(sincos[:, :, :, 0], t_sxyd[:, :, sl(g)])
                        sc_t[g] = sincos
                        if do_outdma:
                            nc.sync.dma_start(oyaw3[:, sl(g), :], yfr_t[g][:, :, 0:T])
                    # S4: sin/cos in place (ACT) of group i-4
                    if 0 <= i - 4 < NGROUPS:
                        g = i - 4
                        scv = sc_t[g].rearrange("p s j f -> p (s j) f")
                        nc.scalar.activation(
                            scv[:, :, 1:FRAME], scv[:, :, 1:FRAME],
                            Act.Sin, scale=SCALE_2PI,
                        )
                    # S5: x/y pscans of group i-5
                    if 0 <= i - 5 < NGROUPS:
                        g = i - 5
                        xfr = p_xfr.tile([P, JG, FRAME], F32, name="xfr")
                        yfr2 = p_yfr2.tile([P, JG, FRAME], F32, name="yfr2")
                        for j in range(JG):
                            nc.vector._custom_dve(
                                PSCAN, out=xfr[:, j, :],
                                in0=sp_st[:, g * JG + j, :],
                                in1=sc_t[g][:, 1, j, :],
                                s0=0.0, s1=0.0, imm2=DT,
                            )
                        for j in range(JG):
                            nc.vector._custom_dve(
                                PSCAN, out=yfr2[:, j, :],
                                in0=sp_st[:, g * JG + j, :],
                                in1=sc_t[g][:, 0, j, :],
                                s0=0.0, s1=0.0, imm2=DT,
                            )
                        xfr_t[g] = xfr
                        yfr2_t[g] = yfr2
                    # S6: x/y out of group i-6
                    if 0 <= i - 6 < NGROUPS and do_outdma:
                        g = i - 6
                        nc.sync.dma_start(ox3[:, sl(g), :], xfr_t[g][:, :, 0:T])
                        nc.sync.dma_start(oy3[:, sl(g), :], yfr2_t[g][:, :, 0:T])
                    # S0: steering prefetch of group i (first 3 pre-issued)
                    if 3 <= i < NGROUPS:
                        g = i
                        stg = p_stg.tile([P, JG, T], F32, name="stg")
                        nc.sync.dma_start(stg[:], st3[:, sl(g), :])
                        stg_t[g] = stg

                if io_internal:
                    nc.sync.dma_start(d_done[:], zt.rearrange("p (a b) -> p a b", a=512)[:, 0, 0:4])

    nc.compile()
    _BUILD_CACHE[key] = nc
    return nc


def kernel(**inputs):
    nc = build_kernel(reps=1)
    inputs = {k: np.asarray(v, dtype=np.float32) for k, v in inputs.items()}
    in_maps = []
    for c in range(N_CORES):
        rows = slice(c * BC, (c + 1) * BC)
        in_maps.append(
            {k: np.ascontiguousarray(inputs[k][rows]) for k in inputs}
        )
    res = run_bass_kernel_spmd(nc, in_maps, list(range(N_CORES))).results
    x = np.concatenate([res[c]["out_x"] for c in range(N_CORES)], axis=0)
    y = np.concatenate([res[c]["out_y"] for c in range(N_CORES)], axis=0)
    yaw = np.concatenate([res[c]["out_yaw"] for c in range(N_CORES)], axis=0)
    sp = np.concatenate([res[c]["out_speed"] for c in range(N_CORES)], axis=0)
    return (x, y, yaw, sp)
